# revision 13
# baseline (speedup 1.0000x reference)
"""Trainium2 Bass kernel for nn_MiniAttentionBlock.

Reference computation (B=16, S=4096, F=512):
    h      = tanh(x @ W + b)        [B,S,F]
    scores = h @ u                  [B,S]
    e      = exp(scores)
    a      = e / (sum(e) + eps)     global normalization over all B*S
    out    = sum_s x[b,s,:] * a[b,s]  -> [B,F]

Strategy: data-parallel over batch, 2 batches per core (8 cores).  The
denominator sum(e) is a single small AllReduce.  Each core receives its
x shard pre-transposed to [F, R] (R = 2*4096 rows) so the contraction
dim F sits on SBUF partitions for the matmul.  x is shipped twice: a
bf16 copy feeding the TensorE (moving-operand bandwidth is byte-limited,
so bf16 doubles matmul throughput vs fp32) and an fp32 copy feeding the
VectorE pooling path (keeps the output at ~2e-4 relative error; score
noise largely cancels between numerator and denominator, but x noise in
the numerator does not).

Per 512-row block:
  - h^T[g, rows] accumulated over 4 k-chunks of bf16 matmuls
  - tanh(+bias) fused on ScalarE, PSUM -> SBUF (bf16 out)
  - scores matmul uses u replicated 128x along the stationary free dim, so
    the PSUM result [128, rows] carries scores broadcast to all partitions
  - exp on ScalarE gives the e-broadcast tile directly (fp32), with the
    block's partial sum accumulated for free (accum_out)
  - weighted pooling sum_rows e*x runs on VectorE: tensor_tensor multiply
    (fp32) then a tensor_scalar pass whose accum_out yields the row-sum
The AllReduce carries the scalar denominator replicated on 128 partitions
([128,1]) so the reciprocal can be applied per-partition with no
partition-broadcast afterwards.
"""

import sys

if "/opt/trn_rl_repo" not in sys.path:
    sys.path.insert(0, "/opt/trn_rl_repo")

import numpy as np
import ml_dtypes

from concourse import bass, bacc, tile, bass_utils
from concourse.dve_ops import TENSOR_TENSOR_REDUCE

mybir = bass.mybir

B, S, F = 16, 4096, 512
N_CORES = 8
BPC = B // N_CORES          # batches per core
R = BPC * S                 # rows per core
RB = 512                    # rows per block
NBLK = R // RB              # blocks per core
DEFER = 8                   # trailing blocks whose pooling overlaps the AllReduce
NKC = F // 128              # 128-partition chunks of F
EPS = 1e-7

F32 = mybir.dt.float32
BF16 = mybir.dt.bfloat16
ALU = mybir.AluOpType
ACTF = mybir.ActivationFunctionType
AXIS = mybir.AxisListType

_CACHE = {}


def _build():
    nc = bacc.Bacc("TRN2", target_bir_lowering=False, debug=False,
                   num_devices=N_CORES)

    xb = nc.dram_tensor("xb", [F, R], BF16, kind="ExternalInput")
    xf = nc.dram_tensor("xf", [F, R], F32, kind="ExternalInput")
    w = nc.dram_tensor("w", [F, F], BF16, kind="ExternalInput")
    b2 = nc.dram_tensor("b2", [128, NKC], F32, kind="ExternalInput")
    ur = nc.dram_tensor("ur", [128, NKC, 128], BF16, kind="ExternalInput")
    out = nc.dram_tensor("out", [BPC, F], F32, kind="ExternalOutput")

    with tile.TileContext(nc) as tc:
        with tc.tile_pool(name="const", bufs=1) as cpool, \
             tc.tile_pool(name="xbp", bufs=4) as xbp, \
             tc.tile_pool(name="xfp", bufs=DEFER + 3) as xfp, \
             tc.tile_pool(name="hap", bufs=10) as hap, \
             tc.tile_pool(name="erp", bufs=DEFER + 2) as erp, \
             tc.tile_pool(name="scr", bufs=3) as scr, \
             tc.tile_pool(name="hps", bufs=6, space="PSUM") as hps, \
             tc.tile_pool(name="sps", bufs=2, space="PSUM") as sps, \
             tc.tile_pool(name="dram", bufs=1, space="DRAM") as dram:

            # ---- constants ----
            w_sb = []
            for kc in range(NKC):
                t = cpool.tile([128, F], BF16, tag=f"w{kc}")
                nc.scalar.dma_start(out=t[:], in_=w.ap()[kc * 128:(kc + 1) * 128, :])
                w_sb.append(t)
            b_sb = cpool.tile([128, NKC], F32, tag="b")
            nc.scalar.dma_start(out=b_sb[:], in_=b2.ap())
            u_sb = cpool.tile([128, NKC, 128], BF16, tag="u")
            nc.scalar.dma_start(out=u_sb[:], in_=ur.ap())

            # warmup collective: pre-warms the ncfw/credit machinery while
            # compute runs; its result is unused
            wu_in = dram.tile([1, 8], F32)
            wu_out = dram.tile([8, 8], F32, addr_space="Shared")
            wu_sb = cpool.tile([1, 8], F32, tag="wusb")
            nc.vector.memset(wu_sb[:], 0.0)
            nc.scalar.dma_start(out=wu_in[:], in_=wu_sb[:])
            nc.gpsimd.collective_compute(
                "AllGather", ALU.bypass,
                replica_groups=[list(range(N_CORES))],
                ins=[wu_in.opt()], outs=[wu_out.opt()])
            ones8 = cpool.tile([8, 128], F32, tag="ones8")
            nc.vector.memset(ones8[:], 1.0)

            esum = cpool.tile([128, NBLK], F32, tag="esum")
            nums = [cpool.tile([128, NBLK], F32, tag=f"num{kc}", name=f"num{kc}")
                    for kc in range(NKC)]
            out_sb = cpool.tile([128, BPC, NKC], F32, tag="osb")

            # DRAM views with the f-chunk index as a free dim: one DMA per
            # block instead of four
            xb_v = xb.ap().rearrange("(c p) r -> p c r", p=128)
            xf_v = xf.ap().rearrange("(c p) r -> p c r", p=128)

            # ---- main loop over row blocks ----
            deferred = []
            pending = None

            def emit_scores_and_pool(blk, hacts, xfs):
                # scores broadcast to all 128 partitions via replicated u
                sp = sps.tile([128, RB], F32, tag="s", name="sp")
                for mc in range(NKC):
                    nc.tensor.matmul(
                        sp[:],
                        lhsT=u_sb[:, mc, :],
                        rhs=hacts[mc][:],
                        start=(mc == 0), stop=(mc == NKC - 1))

                # e (broadcast) = exp(scores); block partial sum for free
                er = erp.tile([128, RB], F32, tag="er", name="er")
                nc.scalar.activation(out=er[:], in_=sp[:], func=ACTF.Exp,
                                     accum_out=esum[:, blk:blk + 1])

                # num[f] += sum_rows xT[f, row] * e[row]  (fp32 path,
                # fused multiply+row-reduce in one custom DVE op)
                if blk < NBLK - DEFER:
                    for kc in range(NKC):
                        sc = scr.tile([128, RB], F32, tag="sc", name="sc")
                        nc.vector._custom_dve(
                            TENSOR_TENSOR_REDUCE,
                            out=sc[:], in0=xfs[kc], in1=er[:],
                            s0=0.0, s1=1.0,
                            accum_out=nums[kc][:, blk:blk + 1])
                else:
                    deferred.append((blk, xfs, er))

            for blk in range(NBLK):
                xball = xbp.tile([128, NKC, RB], BF16, tag="xb", name="xball")
                if blk == 0:
                    # per-chunk loads so the first matmul starts on chunk 0
                    for kc in range(NKC):
                        nc.sync.dma_start(
                            out=xball[:, kc, :],
                            in_=xb_v[:, kc, blk * RB:(blk + 1) * RB])
                else:
                    nc.sync.dma_start(
                        out=xball[:],
                        in_=xb_v[:, :, blk * RB:(blk + 1) * RB])
                xfall = xfp.tile([128, NKC, RB], F32, tag="xf", name="xfall")
                nc.sync.dma_start(
                    out=xfall[:],
                    in_=xf_v[:, :, blk * RB:(blk + 1) * RB])
                xbs = [xball[:, kc, :] for kc in range(NKC)]
                xfs = [xfall[:, kc, :] for kc in range(NKC)]

                # h^T[g, rows] = sum_f W[f, g] * xT[f, rows]; tanh+bias
                hacts = []
                for mc in range(NKC):
                    ps = hps.tile([128, RB], F32, tag="h")
                    for kc in range(NKC):
                        nc.tensor.matmul(
                            ps[:],
                            lhsT=w_sb[kc][:, mc * 128:(mc + 1) * 128],
                            rhs=xbs[kc],
                            start=(kc == 0), stop=(kc == NKC - 1))
                    ha = hap.tile([128, RB], BF16, tag="h")
                    nc.scalar.activation(out=ha[:], in_=ps[:], func=ACTF.Tanh,
                                         bias=b_sb[:, mc:mc + 1])
                    hacts.append(ha)

                # scores/exp/pooling of the PREVIOUS block, so the tanh of
                # this block's last chunk never stalls the TensorE
                if pending is not None:
                    emit_scores_and_pool(*pending)
                pending = (blk, hacts, xfs)

            emit_scores_and_pool(*pending)

            # ---- finalize ----
            # local sum, replicated on all 128 partitions
            s_loc = cpool.tile([128, 1], F32, tag="sloc")
            nc.vector.tensor_reduce(out=s_loc[:], in_=esum[:],
                                    axis=AXIS.X, op=ALU.add)

            cc_in = dram.tile([1, 8], F32)
            cc_out = dram.tile([8, 8], F32, addr_space="Shared")
            s8 = cpool.tile([1, 8], F32, tag="s8")
            nc.vector.tensor_scalar_add(out=s8[:], in0=wu_sb[:],
                                        scalar1=s_loc[0:1, 0:1])
            nc.sync.dma_start(out=cc_in[:], in_=s8[:])
            nc.gpsimd.collective_compute(
                "AllGather", ALU.bypass,
                replica_groups=[list(range(N_CORES))],
                ins=[cc_in.opt()], outs=[cc_out.opt()])
            sg8 = cpool.tile([8, 8], F32, tag="sg8")
            nc.sync.dma_start(out=sg8[:], in_=cc_out[:])
            # combine the 8 gathered partials and broadcast to 128 partitions
            # in one small fp32 matmul: ones8^T[128,8] @ sg8[:,0] -> [128,1]
            psg = sps.tile([128, 1], F32, tag="s")
            nc.tensor.matmul(psg[:], lhsT=ones8[:], rhs=sg8[:, 0:1],
                             start=True, stop=True)
            sg = cpool.tile([128, 1], F32, tag="sg")
            nc.scalar.copy(out=sg[:], in_=psg[:])

            # pooling for the deferred blocks, overlapping the AllReduce
            for blk, xfs, er in deferred:
                for kc in range(NKC):
                    sc = scr.tile([128, RB], F32, tag="sc")
                    nc.vector._custom_dve(
                        TENSOR_TENSOR_REDUCE,
                        out=sc[:], in0=xfs[kc], in1=er[:],
                        s0=0.0, s1=1.0,
                        accum_out=nums[kc][:, blk:blk + 1])

            rcp = cpool.tile([128, 1], F32, tag="rcp")
            nc.vector.tensor_scalar_add(out=rcp[:], in0=sg[:], scalar1=EPS)
            nc.vector.reciprocal(out=rcp[:], in_=rcp[:])

            for bb in range(BPC):
                for kc in range(NKC):
                    nc.vector.tensor_reduce(
                        out=out_sb[:, bb, kc:kc + 1],
                        in_=nums[kc][:, bb * (NBLK // BPC):(bb + 1) * (NBLK // BPC)],
                        axis=AXIS.X, op=ALU.add)
            nc.vector.tensor_scalar_mul(out=out_sb[:], in0=out_sb[:],
                                        scalar1=rcp[:])

            nc.sync.dma_start(
                out=out.ap().rearrange("b (c p) -> p b c", p=128),
                in_=out_sb[:])

    nc.compile()
    return nc


def _get_compiled():
    if "nc" not in _CACHE:
        _CACHE["nc"] = _build()
    return _CACHE["nc"]


def _make_in_maps(x, W, b, u):
    Wc = np.ascontiguousarray(
        np.asarray(W, np.float32).astype(ml_dtypes.bfloat16))
    bc = np.ascontiguousarray(np.asarray(b, np.float32).reshape(NKC, 128).T)
    u_cols = np.asarray(u, np.float32).reshape(NKC, 128).T  # [128, NKC]
    urc = np.ascontiguousarray(
        np.broadcast_to(u_cols[:, :, None], (128, NKC, 128))
    ).astype(ml_dtypes.bfloat16)
    in_maps = []
    for c in range(N_CORES):
        xs = np.ascontiguousarray(
            np.asarray(x[BPC * c:BPC * (c + 1)], np.float32).reshape(R, F).T)
        in_maps.append({"xb": xs.astype(ml_dtypes.bfloat16),
                        "xf": xs, "w": Wc, "b2": bc, "ur": urc})
    return in_maps


def kernel(x, W, b, u):
    nc = _get_compiled()
    in_maps = _make_in_maps(x, W, b, u)
    res = bass_utils.run_bass_kernel_spmd(
        nc, in_maps, core_ids=list(range(N_CORES)))
    _CACHE["last_results"] = res
    return np.concatenate([res.results[c]["out"] for c in range(N_CORES)],
                          axis=0)


def kernel_traced(x, W, b, u, **trace_kwargs):
    """Same as kernel() but with NTFF tracing; returns (out, BassKernelResults)."""
    nc = _get_compiled()
    in_maps = _make_in_maps(x, W, b, u)
    res = bass_utils.run_bass_kernel_spmd(
        nc, in_maps, core_ids=list(range(N_CORES)), trace=True, **trace_kwargs)
    _CACHE["last_results"] = res
    out = np.concatenate([res.results[c]["out"] for c in range(N_CORES)],
                         axis=0)
    return out, res
